# revision 2
# baseline (speedup 1.0000x reference)
"""Trainium2 Bass kernel for sliding-window unfold (im2col).

reference:  out = x[:, idx, :]  with idx[w, f] = w + f
  x:   [128, 4096, 4]  f32
  out: [128, 4065, 32, 4]  f32

Key structural fact: out[b, w] (= 32*4 = 128 floats = 512 B) is the
contiguous slice x[b].flat[4w : 4w + 128].  The whole problem is a
sliding-window byte replication; HBM write bandwidth is the roofline.
Per core (16 batches): 33.3 MB of output writes; measured SWDGE store
streams sustain ~425-430 GB/s when the descriptor queue stays deep, so
the floor is ~78 us of store drain + ~10 us fixed framework pre/post.

Per-core structure (pure data parallel, batch 128 -> 16 per core):
  bulk: partition p holds windows 31p..31p+30 of one batch.
    load  X[128, 248/batch]  - everything those windows touch
    expand X -> Y[128, 3968] (overlapping-stride read AP) on ACT/DVE
    store Y -> out[b] windows 0..3967 (contiguous 15.5 KB/partition)
  tail: windows 3937..4064 of ALL 16 batches ride ONE load + ONE store
    (one 512 B window per partition per batch; the first 31 windows per
    batch rewrite bulk output with identical bytes).

Scheduling (what the previous iteration got wrong): every dma_start
trigger costs ~0.6 us on its issuing engine and GPSIMD's stream is
in-order, so per-batch tail stores issued up front blocked the bulk
stores behind 16 separate tail-load waits, idling the SDMA engines
from ~13-25 us.  Now:
  - sync (HWDGE) : 4 grouped X loads (4 batches, 508 KB each), then
    the single coalesced tail store (descriptors generated in RTL, so
    the 2048 512 B chunks cost no GPSIMD time).
  - scalar (ACT) : the 1 MB tail load first, then 8 expands.
  - vector (DVE) : 8 expands (1-port mode; never locks GPSIMD out).
  - gpsimd (SWDGE): the 16 bulk stores only, in completion order -
    a pure store queue that keeps all 16 SDMA engines saturated.
"""

import numpy as np

from concourse import bacc, mybir, tile
from concourse.bass_utils import run_bass_kernel_spmd

N_CORES = 8
B_FULL = 128
B = B_FULL // N_CORES  # 16 batches per core
S = 4096
C = 4
F = 32
W = S - F + 1    # 4065
FL = F * C       # 128 floats per window
XB = S * C       # 16384 floats per batch of x
OB = W * FL      # 520320 floats per batch of out
WPP = 31         # windows per partition in the bulk store
NBULK = 128 * WPP          # 3968 bulk windows per batch
NTAIL = W - NBULK          # 97 tail windows
YROW = WPP * FL            # 3968 floats per partition row
XROW = (WPP - 1) * C + FL  # 248 floats of x per partition per batch
GB = 4                     # batches per grouped X load
NG = B // GB               # 4 groups
XGROW = GB * XROW          # 992 floats per partition per group

_cache = {}


def build_nc():
    nc = bacc.Bacc("TRN2", target_bir_lowering=False)
    x = nc.dram_tensor("x", [B, S, C], mybir.dt.float32, kind="ExternalInput")
    out = nc.dram_tensor("out", [B, W, F, C], mybir.dt.float32, kind="ExternalOutput")

    with tile.TileContext(nc) as tc:
        with (
            tc.tile_pool(name="xp", bufs=NG) as xp,
            tc.tile_pool(name="yp", bufs=10) as yp,
            tc.tile_pool(name="tp", bufs=1) as tp,
        ):
            # tail load: T[p, 128b+i] = x[b].flat[4*(3937+p) + i]
            # (window 3937+p of batch b, materialized by an overlapping
            # 512 B-per-window read).  One 1 MB DMA on the ACT HWDGE ring.
            T = tp.tile([128, B * FL], mybir.dt.float32)
            srcT = x[:].copy()
            srcT.ap = mybir.VecI64Pair([[C, 128], [XB, B], [1, FL]])
            srcT.offset = (NBULK - 31) * C
            dstT = T[:].copy()
            dstT.ap = mybir.VecI64Pair([[B * FL, 128], [FL, B], [1, FL]])
            dstT.offset = 0
            nc.scalar.dma_start(out=dstT, in_=srcT)

            # grouped bulk loads: XG[p, 248j+i] = x[4g+j].flat[124p + i]
            XG = []
            for g in range(NG):
                Xt = xp.tile([128, XGROW], mybir.dt.float32)
                src = x[:].copy()
                src.ap = mybir.VecI64Pair([[WPP * C, 128], [XB, GB], [1, XROW]])
                src.offset = g * GB * XB
                dst = Xt[:].copy()
                dst.ap = mybir.VecI64Pair([[XGROW, 128], [XROW, GB], [1, XROW]])
                dst.offset = 0
                nc.sync.dma_start(out=dst, in_=src)
                XG.append(Xt)

            # coalesced tail store on the sync HWDGE ring: RTL descriptor
            # generation keeps the 2048 small chunks off GPSIMD's stream.
            dstTs = out[:].copy()
            dstTs.ap = mybir.VecI64Pair([[FL, 128], [OB, B], [1, FL]])
            dstTs.offset = (NBULK - 31) * FL
            srcTs = T[:].copy()
            srcTs.ap = mybir.VecI64Pair([[B * FL, 128], [FL, B], [1, FL]])
            srcTs.offset = 0
            nc.sync.dma_start(out=dstTs, in_=srcTs)

            # per batch: expand (ACT for even, DVE for odd) then bulk store
            for b in range(B):
                g, j = divmod(b, GB)
                Y = yp.tile([128, YROW], mybir.dt.float32)
                src2 = XG[g][:].copy()
                src2.ap = mybir.VecI64Pair([[XGROW, 128], [C, WPP], [1, FL]])
                src2.offset = j * XROW
                dst2 = Y[:].copy()
                dst2.ap = mybir.VecI64Pair([[YROW, 128], [FL, WPP], [1, FL]])
                dst2.offset = 0
                if b % 2 == 0:
                    nc.scalar.copy(out=dst2, in_=src2)
                else:
                    nc.vector.tensor_copy(out=dst2, in_=src2)

                dst3 = out[:].copy()
                dst3.ap = mybir.VecI64Pair([[YROW, 128], [1, YROW]])
                dst3.offset = b * OB
                nc.gpsimd.dma_start(out=dst3, in_=Y[:, :])

    nc.finalize()
    return nc


def run_sharded(x: np.ndarray, trace: bool = False):
    """Shard batch across 8 cores, run, gather. Returns (out, raw results)."""
    if "nc" not in _cache:
        _cache["nc"] = build_nc()
    nc = _cache["nc"]

    x = np.ascontiguousarray(x, dtype=np.float32)
    in_maps = [{"x": x[i * B : (i + 1) * B]} for i in range(N_CORES)]
    res = run_bass_kernel_spmd(nc, in_maps, list(range(N_CORES)), trace=trace)
    out = np.concatenate([res.results[i]["out"] for i in range(N_CORES)], axis=0)
    return out, res


def kernel(x: np.ndarray) -> np.ndarray:
    out, _ = run_sharded(x, trace=False)
    return out


# revision 9
# speedup vs baseline: 1.0974x; 1.0974x over previous
"""Trainium2 Bass kernel for sliding-window unfold (im2col).

reference:  out = x[:, idx, :]  with idx[w, f] = w + f
  x:   [128, 4096, 4]  f32
  out: [128, 4065, 32, 4]  f32

out[b, w] (= 128 floats = 512 B) is the contiguous slice
x[b].flat[4w : 4w + 128]; the problem is a sliding-window byte
replication and HBM write bandwidth is the roofline.  Per core
(16 batches): 33.3 MB of output writes.  A deep SWDGE store queue
sustains ~420-430 GB/s, so the floor is ~79 us of store drain plus
~10 us fixed framework pre/post-amble.

Hard-won scheduling facts (from NTFF traces of prior iterations):
  - DGE descriptor generation is serial per ring at ~5 ns/desc, so a
    DMA built of 512 B chunks tops out near ~100 GB/s.  Descriptor
    size, not DMA count, is what matters.
  - Tile inserts a WAW semaphore between DMAs whose DRAM byte ranges
    overlap: a tail store that rewrites bulk windows blocks EVERY
    bulk store behind its completion.  Tail and bulk must be disjoint.
  - Every dma_start trigger costs ~0.6 us on its issuing engine, and
    engine streams are in-order; keep GPSIMD's stream stores-only.

Layout (per core):
  bulk: partition p holds windows 31p..31p+30 of one batch b.
    load X (248 f32/partition/batch), expand on ACT/DVE into
    Y[128, 3968] via an overlapping-stride read AP, store Y ->
    out[b] windows 0..3967 (contiguous 15.5 KB per partition ->
    128 fat descriptors).  Batch 0's expand is split in half across
    DVE and ACT so the first store issues at ~8.5 us.
  tail: windows 3968..4064 (exactly; disjoint from bulk).  Partition
    p = 8b+s holds 12 consecutive windows 12s..12s+11 of batch b's
    tail, expanded from a tiny raw load; consecutive windows are
    contiguous in out, so store descriptors are 6 KB and all writes
    are strictly disjoint.  Window 4064 is contiguous x data and
    rides a raw 64 B-chunk load+store pair.
"""

import numpy as np

from concourse import bacc, mybir, tile
from concourse.bass_utils import run_bass_kernel_spmd

N_CORES = 8
B_FULL = 128
B = B_FULL // N_CORES  # 16 batches per core
S = 4096
C = 4
F = 32
W = S - F + 1    # 4065
FL = F * C       # 128 floats per window
XB = S * C       # 16384 floats per batch of x
OB = W * FL      # 520320 floats per batch of out
WPP = 31         # windows per partition in the bulk store
NBULK = 128 * WPP          # 3968 bulk windows per batch
YROW = WPP * FL            # 3968 floats per partition row
XROW = (WPP - 1) * C + FL  # 248 floats of x per partition per batch

# tail geometry: windows 3968..4063 as 8 slices of 12 windows per batch
# (partition p = 8*b + s, strictly disjoint writes); window 4064 rides
# a separate raw load+store of 64 B chunks (it IS contiguous x data).
TSL = 8                    # slices per batch
TWIN = 12                  # windows per slice
TSTR = 12                  # window stride between slices
TROW = TWIN * FL           # 1536 floats of tail output per partition
RROW = (TWIN - 1) * C + FL # 172 floats of raw x used per partition
RLD = 176                  # floats loaded (slice 7 runs to end of batch)
W4 = W - 1                 # window 4064
W4C = FL // TSL            # 16 floats of window 4064 per partition
H0 = 16                    # windows in bulk batch-0 first half
H1 = WPP - H0              # 15 windows in second half

_cache = {}


def build_nc():
    nc = bacc.Bacc("TRN2", target_bir_lowering=False)
    x = nc.dram_tensor("x", [B, S, C], mybir.dt.float32, kind="ExternalInput")
    out = nc.dram_tensor("out", [B, W, F, C], mybir.dt.float32, kind="ExternalOutput")

    with tile.TileContext(nc) as tc:
        with (
            tc.tile_pool(name="x01", bufs=2) as x01,
            tc.tile_pool(name="xg1", bufs=1) as xg1p,
            tc.tile_pool(name="xg2", bufs=1) as xg2p,
            tc.tile_pool(name="y0a", bufs=1) as y0ap,
            tc.tile_pool(name="y0b", bufs=1) as y0bp,
            tc.tile_pool(name="yp", bufs=10) as yp,
            tc.tile_pool(name="rp", bufs=2) as rp,
            tc.tile_pool(name="tp", bufs=1) as tp,
        ):
            def ld(engine, dst_tile, dst_ap, dst_off, src_ap, src_off):
                src = x[:].copy()
                src.ap = mybir.VecI64Pair(src_ap)
                src.offset = src_off
                dst = dst_tile[:].copy()
                dst.ap = mybir.VecI64Pair(dst_ap)
                dst.offset = dst_off
                engine.dma_start(out=dst, in_=src)

            def st(engine, src_tile, src_ap, src_off, dst_ap, dst_off):
                dst = out[:].copy()
                dst.ap = mybir.VecI64Pair(dst_ap)
                dst.offset = dst_off
                src = src_tile[:].copy()
                src.ap = mybir.VecI64Pair(src_ap)
                src.offset = src_off
                engine.dma_start(out=dst, in_=src)

            def expand(engine, src_tile, src_row, src_off, dst_tile, dst_row, nwin):
                src = src_tile[:].copy()
                src.ap = mybir.VecI64Pair([[src_row, 128], [C, nwin], [1, FL]])
                src.offset = src_off
                dst = dst_tile[:].copy()
                dst.ap = mybir.VecI64Pair([[dst_row, 128], [FL, nwin], [1, FL]])
                dst.offset = 0
                if engine is nc.vector:
                    engine.tensor_copy(out=dst, in_=src)
                else:
                    engine.copy(out=dst, in_=src)

            # ---- loads ----
            # sync ring: batch 0, batch 1, batches 2..7
            X0 = x01.tile([128, XROW], mybir.dt.float32)
            ld(nc.sync, X0, [[XROW, 128], [1, XROW]], 0,
               [[WPP * C, 128], [1, XROW]], 0)
            X1 = x01.tile([128, XROW], mybir.dt.float32)
            ld(nc.sync, X1, [[XROW, 128], [1, XROW]], 0,
               [[WPP * C, 128], [1, XROW]], XB)
            XG1 = xg1p.tile([128, 6 * XROW], mybir.dt.float32)
            ld(nc.sync, XG1, [[6 * XROW, 128], [XROW, 6], [1, XROW]], 0,
               [[WPP * C, 128], [XB, 6], [1, XROW]], 2 * XB)
            # scalar ring: tail raw load, window-4064 load, batches 8..15
            R = rp.tile([128, RLD], mybir.dt.float32)
            ld(nc.scalar, R, [[RLD, 128], [1, RLD]], 0,
               [[XB, B], [TSTR * C, TSL], [1, RLD]], NBULK * C)
            V = rp.tile([128, W4C], mybir.dt.float32)
            ld(nc.scalar, V, [[W4C, 128], [1, W4C]], 0,
               [[XB, B], [W4C, TSL], [1, W4C]], W4 * C)
            XG2 = xg2p.tile([128, 8 * XROW], mybir.dt.float32)
            ld(nc.scalar, XG2, [[8 * XROW, 128], [XROW, 8], [1, XROW]], 0,
               [[WPP * C, 128], [XB, 8], [1, XROW]], 8 * XB)

            # ---- expands ----
            # DVE: batch-0 first half, tail expand, then odd batches
            Y0a = y0ap.tile([128, H0 * FL], mybir.dt.float32)
            expand(nc.vector, X0, XROW, 0, Y0a, H0 * FL, H0)
            T = tp.tile([128, TROW], mybir.dt.float32)
            expand(nc.vector, R, RLD, 0, T, TROW, TWIN)
            # ACT: batch-0 second half, then even batches
            Y0b = y0bp.tile([128, H1 * FL], mybir.dt.float32)
            expand(nc.scalar, X0, XROW, H0 * C, Y0b, H1 * FL, H1)

            Ys = {}
            for b in range(1, B):
                eng = nc.vector if b % 2 == 1 else nc.scalar
                if b == 1:
                    src_t, row, off = X1, XROW, 0
                elif b < 8:
                    src_t, row, off = XG1, 6 * XROW, (b - 2) * XROW
                else:
                    src_t, row, off = XG2, 8 * XROW, (b - 8) * XROW
                Y = yp.tile([128, YROW], mybir.dt.float32)
                expand(eng, src_t, row, off, Y, YROW, WPP)
                Ys[b] = Y

            # tail stores ride the sync ring once the tail expand lands;
            # dst ranges [3968*128, OB) per batch are disjoint from bulk
            # and from each other.
            st(nc.sync, T, [[TROW, 128], [1, TROW]], 0,
               [[OB, B], [TSTR * FL, TSL], [1, TROW]], NBULK * FL)
            st(nc.sync, V, [[W4C, 128], [1, W4C]], 0,
               [[OB, B], [W4C, TSL], [1, W4C]], W4 * FL)

            # ---- bulk stores (GPSIMD / SWDGE only, completion order) ----
            st(nc.gpsimd, Y0a, [[H0 * FL, 128], [1, H0 * FL]], 0,
               [[YROW, 128], [1, H0 * FL]], 0)
            st(nc.gpsimd, Y0b, [[H1 * FL, 128], [1, H1 * FL]], 0,
               [[YROW, 128], [1, H1 * FL]], H0 * FL)
            order = [2, 1, 4, 3, 6, 5, 8, 7, 10, 9, 12, 11, 14, 13, 15]
            for b in order:
                st(nc.gpsimd, Ys[b], [[YROW, 128], [1, YROW]], 0,
                   [[YROW, 128], [1, YROW]], b * OB)

    nc.finalize()
    return nc


def run_sharded(x: np.ndarray, trace: bool = False):
    """Shard batch across 8 cores, run, gather. Returns (out, raw results)."""
    if "nc" not in _cache:
        _cache["nc"] = build_nc()
    nc = _cache["nc"]

    x = np.ascontiguousarray(x, dtype=np.float32)
    in_maps = [{"x": x[i * B : (i + 1) * B]} for i in range(N_CORES)]
    res = run_bass_kernel_spmd(nc, in_maps, list(range(N_CORES)), trace=trace)
    out = np.concatenate([res.results[i]["out"] for i in range(N_CORES)], axis=0)
    return out, res


def kernel(x: np.ndarray) -> np.ndarray:
    out, _ = run_sharded(x, trace=False)
    return out


# revision 11
# speedup vs baseline: 1.1751x; 1.0708x over previous
"""Trainium2 Bass kernel for sliding-window unfold (im2col).

reference:  out = x[:, idx, :]  with idx[w, f] = w + f
  x:   [128, 4096, 4]  f32
  out: [128, 4065, 32, 4]  f32

out[b, w] (= 128 floats = 512 B) is the contiguous slice
x[b].flat[4w : 4w + 128]; the problem is a sliding-window byte
replication and HBM write bandwidth is the roofline.  Per core
(16 batches): 33.3 MB of output writes.  A deep SWDGE store queue
sustains ~420-430 GB/s, so the floor is ~79 us of store drain plus
~10 us fixed framework pre/post-amble.

Hard-won scheduling facts (from NTFF traces of prior iterations):
  - DGE descriptor generation is serial per ring at ~5 ns/desc, so a
    DMA built of 512 B chunks tops out near ~100 GB/s.  Descriptor
    size, not DMA count, is what matters.
  - Tile inserts a WAW semaphore between DMAs whose DRAM byte ranges
    overlap: a tail store that rewrites bulk windows blocks EVERY
    bulk store behind its completion.  Tail and bulk must be disjoint.
  - Every dma_start trigger costs ~0.6 us on its issuing engine, and
    engine streams are in-order; keep GPSIMD's stream stores-only.

Layout (per core):
  bulk: partition p holds windows 31p..31p+30 of one batch b.
    load X (248 f32/partition/batch), expand on ACT/DVE into
    Y[128, 3968] via an overlapping-stride read AP, store Y ->
    out[b] windows 0..3967 (contiguous 15.5 KB per partition ->
    128 fat descriptors).  Batch 0's expand is split in half across
    DVE and ACT so the first store issues at ~8.5 us.
  tail: windows 3968..4064 (exactly; disjoint from bulk).  Partition
    p = 8b+s holds 12 consecutive windows 12s..12s+11 of batch b's
    tail, expanded from a tiny raw load; consecutive windows are
    contiguous in out, so store descriptors are 6 KB and all writes
    are strictly disjoint.  Window 4064 is contiguous x data and
    rides a raw 64 B-chunk load+store pair.
"""

import numpy as np

from concourse import bacc, mybir, tile
from concourse.bass_utils import run_bass_kernel_spmd

N_CORES = 8
B_FULL = 128
B = B_FULL // N_CORES  # 16 batches per core
S = 4096
C = 4
F = 32
W = S - F + 1    # 4065
FL = F * C       # 128 floats per window
XB = S * C       # 16384 floats per batch of x
OB = W * FL      # 520320 floats per batch of out
WPP = 31         # windows per partition in the bulk store
NBULK = 128 * WPP          # 3968 bulk windows per batch
YROW = WPP * FL            # 3968 floats per partition row
XROW = (WPP - 1) * C + FL  # 248 floats of x per partition per batch

# tail geometry: windows 3968..4063 as 8 slices of 12 windows per batch
# (partition p = 8*b + s, strictly disjoint writes); window 4064 rides
# a separate raw load+store of 64 B chunks (it IS contiguous x data).
TSL = 8                    # slices per batch
TWIN = 12                  # windows per slice
TSTR = 12                  # window stride between slices
TROW = TWIN * FL           # 1536 floats of tail output per partition
RROW = (TWIN - 1) * C + FL # 172 floats of raw x used per partition
RLD = 176                  # floats loaded (slice 7 runs to end of batch)
W4 = W - 1                 # window 4064
W4C = FL // TSL            # 16 floats of window 4064 per partition
H0 = 16                    # windows in bulk batch-0 first half
H1 = WPP - H0              # 15 windows in second half

_cache = {}


def build_nc():
    nc = bacc.Bacc("TRN2", target_bir_lowering=False)
    x = nc.dram_tensor("x", [B, S, C], mybir.dt.float32, kind="ExternalInput")
    out = nc.dram_tensor("out", [B, W, F, C], mybir.dt.float32, kind="ExternalOutput")

    with tile.TileContext(nc) as tc:
        with (
            tc.tile_pool(name="x01", bufs=2) as x01,
            tc.tile_pool(name="xg1", bufs=1) as xg1p,
            tc.tile_pool(name="xg2", bufs=1) as xg2p,
            tc.tile_pool(name="y0a", bufs=1) as y0ap,
            tc.tile_pool(name="y0b", bufs=1) as y0bp,
            tc.tile_pool(name="yp", bufs=10) as yp,
            tc.tile_pool(name="rp", bufs=2) as rp,
            tc.tile_pool(name="tp", bufs=1) as tp,
        ):
            def ld(engine, dst_tile, dst_ap, dst_off, src_ap, src_off):
                src = x[:].copy()
                src.ap = mybir.VecI64Pair(src_ap)
                src.offset = src_off
                dst = dst_tile[:].copy()
                dst.ap = mybir.VecI64Pair(dst_ap)
                dst.offset = dst_off
                engine.dma_start(out=dst, in_=src)

            def st(engine, src_tile, src_ap, src_off, dst_ap, dst_off):
                dst = out[:].copy()
                dst.ap = mybir.VecI64Pair(dst_ap)
                dst.offset = dst_off
                src = src_tile[:].copy()
                src.ap = mybir.VecI64Pair(src_ap)
                src.offset = src_off
                engine.dma_start(out=dst, in_=src)

            def expand(engine, src_tile, src_row, src_off, dst_tile, dst_row, nwin):
                src = src_tile[:].copy()
                src.ap = mybir.VecI64Pair([[src_row, 128], [C, nwin], [1, FL]])
                src.offset = src_off
                dst = dst_tile[:].copy()
                dst.ap = mybir.VecI64Pair([[dst_row, 128], [FL, nwin], [1, FL]])
                dst.offset = 0
                if engine is nc.vector:
                    engine.tensor_copy(out=dst, in_=src)
                else:
                    engine.copy(out=dst, in_=src)

            # ---- loads ----
            # sync ring: batch 0, batch 1, batches 2..7
            X0 = x01.tile([128, XROW], mybir.dt.float32)
            ld(nc.sync, X0, [[XROW, 128], [1, XROW]], 0,
               [[WPP * C, 128], [1, XROW]], 0)
            X1 = x01.tile([128, XROW], mybir.dt.float32)
            ld(nc.sync, X1, [[XROW, 128], [1, XROW]], 0,
               [[WPP * C, 128], [1, XROW]], XB)
            XG1 = xg1p.tile([128, 6 * XROW], mybir.dt.float32)
            ld(nc.sync, XG1, [[6 * XROW, 128], [XROW, 6], [1, XROW]], 0,
               [[WPP * C, 128], [XB, 6], [1, XROW]], 2 * XB)
            # scalar ring: tail raw load, window-4064 load, batches 8..15
            R = rp.tile([128, RLD], mybir.dt.float32)
            ld(nc.scalar, R, [[RLD, 128], [1, RLD]], 0,
               [[XB, B], [TSTR * C, TSL], [1, RLD]], NBULK * C)
            V = rp.tile([128, W4C], mybir.dt.float32)
            ld(nc.scalar, V, [[W4C, 128], [1, W4C]], 0,
               [[XB, B], [W4C, TSL], [1, W4C]], W4 * C)
            XG2 = xg2p.tile([128, 8 * XROW], mybir.dt.float32)
            ld(nc.scalar, XG2, [[8 * XROW, 128], [XROW, 8], [1, XROW]], 0,
               [[WPP * C, 128], [XB, 8], [1, XROW]], 8 * XB)

            # ---- expands ----
            # DVE: batch-0 first half, then odd batches (a 2-port-mode
            # DVE copy locks GPSIMD out of the shared SBUF port and
            # stalls SWDGE descriptor emission, so keep DVE's queue
            # clear right when the first stores are being emitted).
            Y0a = y0ap.tile([128, H0 * FL], mybir.dt.float32)
            expand(nc.vector, X0, XROW, 0, Y0a, H0 * FL, H0)
            # ACT: batch-0 second half, tail expand, then even batches
            Y0b = y0bp.tile([128, H1 * FL], mybir.dt.float32)
            expand(nc.scalar, X0, XROW, H0 * C, Y0b, H1 * FL, H1)
            T = tp.tile([128, TROW], mybir.dt.float32)
            expand(nc.scalar, R, RLD, 0, T, TROW, TWIN)

            Ys = {}
            for b in range(1, B):
                eng = nc.vector if b % 2 == 1 else nc.scalar
                if b == 1:
                    src_t, row, off = X1, XROW, 0
                elif b < 8:
                    src_t, row, off = XG1, 6 * XROW, (b - 2) * XROW
                else:
                    src_t, row, off = XG2, 8 * XROW, (b - 8) * XROW
                Y = yp.tile([128, YROW], mybir.dt.float32)
                expand(eng, src_t, row, off, Y, YROW, WPP)
                Ys[b] = Y

            # ---- stores: ALL on GPSIMD/SWDGE.  HWDGE stores measurably
            # degrade SDMA engine 15 (~21 vs 26.5 GB/s for the whole
            # overlap window), unbalancing the drain; with SWDGE-only
            # stores the baseline showed all 16 engines balanced.
            st(nc.gpsimd, Y0a, [[H0 * FL, 128], [1, H0 * FL]], 0,
               [[YROW, 128], [1, H0 * FL]], 0)
            st(nc.gpsimd, Y0b, [[H1 * FL, 128], [1, H1 * FL]], 0,
               [[YROW, 128], [1, H1 * FL]], H0 * FL)
            for b in range(1, B):
                st(nc.gpsimd, Ys[b], [[YROW, 128], [1, YROW]], 0,
                   [[YROW, 128], [1, YROW]], b * OB)
            # tail stores last: small, their expands landed long ago;
            # dst ranges [3968*128, OB) per batch are disjoint from bulk
            # and from each other.
            st(nc.gpsimd, T, [[TROW, 128], [1, TROW]], 0,
               [[OB, B], [TSTR * FL, TSL], [1, TROW]], NBULK * FL)
            st(nc.gpsimd, V, [[W4C, 128], [1, W4C]], 0,
               [[OB, B], [W4C, TSL], [1, W4C]], W4 * FL)

    nc.finalize()
    return nc


def run_sharded(x: np.ndarray, trace: bool = False):
    """Shard batch across 8 cores, run, gather. Returns (out, raw results)."""
    if "nc" not in _cache:
        _cache["nc"] = build_nc()
    nc = _cache["nc"]

    x = np.ascontiguousarray(x, dtype=np.float32)
    in_maps = [{"x": x[i * B : (i + 1) * B]} for i in range(N_CORES)]
    res = run_bass_kernel_spmd(nc, in_maps, list(range(N_CORES)), trace=trace)
    out = np.concatenate([res.results[i]["out"] for i in range(N_CORES)], axis=0)
    return out, res


def kernel(x: np.ndarray) -> np.ndarray:
    out, _ = run_sharded(x, trace=False)
    return out
